# revision 16
# baseline (speedup 1.0000x reference)
"""Trainium2 kernel for nn_H100SmartEmbedding (embedding_lookup).

Output [131072, 768] f32: cols 0:128 price_w[0] (const), 128:256 size_w[0]
(const), 256:384 exchange_w[i%3], 384:512 pair_w[i%7], 512:640 level_w[i%15],
640:768 time_w[i%31].  Rows repeat with period lcm(3,7,15,31)=3255.

Each of the 8 cores covers 16384 output rows.  The core builds one period
block (3328 rows = 128 partitions x 26 rows) in SBUF and replicates it to
DRAM with large contiguous writes; the ~48 MiB/core write is the memory
roofline (~25.5 GB/s x 16 SDMA engines after the 2-packets-per-79KB-line
turnaround tax).  Tables are bf16 (single component, rel err ~4e-3 vs the
2e-2 gate), so all six tables stack block-diagonally into K=12
(const/exch/pair) and K=48 (level/time) contraction rows: two matmuls +
two PSUM->SBUF copies per chunk, copies alternating vector/scalar per
chunk.  Hard-won constraints baked in here:
 - every big DMA covers all 128 partitions (a 121-partition DMA lands on
   just 11 SDMA engines and halves write throughput);
 - one HWDGE queue only (two concurrent queues collapsed throughput);
 - a PSUM region is copied only after one LATER matmul also completed --
   a copy released directly on its own matmul's completion semaphore
   races the PSUM write drain and intermittently reads zeros;
 - the 36 wrap tail rows go out as 3 tiny early-issued DMAs absorbed
   while the big replicas still queue.
"""

import sys

if "/opt/trn_rl_repo" not in sys.path:
    sys.path.insert(0, "/opt/trn_rl_repo")

import numpy as np

N = 131072
D = 768
PERIOD = 3255  # lcm(3, 7, 15, 31)
NCORES = 8
RPC = N // NCORES  # 16384 rows per core
CHUNK = 26  # rows per SBUF partition
NREPS = 5  # replica bases k*PERIOD, k=0..4
BROWS = CHUNK * 128  # 3328 staged rows (period + wrap-correct padding)
# Every big write uses all 128 partitions: a partial-partition DMA is
# spread over fewer SDMA engines (121 partitions -> 11 engines) and tanks
# HBM write throughput.  Replicas land at k*PERIOD writing the full block;
# rows 16348:16384 (block rows 73..108 by wrap) come from 3 tiny
# single-partition tail DMAs issued EARLY so their packets drain while the
# big replicas still queue behind them.
OUT_ROWS = RPC  # 16384, no padding
# Write groups of 2 chunks: production (~1.7us/pair) slightly outpaces the
# DMA service time per pair, so the write queue stays non-empty through the
# whole fill phase.
GROUPS = [(q0, 2) for q0 in range(0, CHUNK, 2)]

# packed input tensor pk [48, PK_F] bf16 free-dim layout:
#   [0:512]    tabsA rows 0:12 (price|size|exchange|pair block-diag)
#   [512:768]  tabsB rows 0:48 (level|time block-diag)
#   then one 512-col block per chunk pair j (chunks 2j, 2j+1):
#     [base + (q%2)*128]        ohA cols of chunk q  (rows 0:12)
#     [base + 256 + (q%2)*128]  ohB cols of chunk q  (rows 0:48)
# The first load slice [0:L1_COLS] unlocks chunks 0-1, the rest arrives in
# a second DMA that overlaps the first matmuls.
PK_F = 768 + 512 * (CHUNK // 2)  # 7424
L1_COLS = 1280


def _oha_off(q):
    return 768 + 512 * (q // 2) + (q % 2) * 128


def _ohb_off(q):
    return 768 + 512 * (q // 2) + 256 + (q % 2) * 128

TRACE = False
LAST_EXEC_NS = None
LAST_RESULT = None

_nc_cache = {}


def _ensure_ntff_hook():
    """The agent image's antenv package lacks axon_hooks, so the boot shim
    never registers the NTFF profile hook and trace=True crashes on import.
    Recreate the module + ctypes hook here (same recipe as trn_boot.py)."""
    import types
    import ctypes
    import contextlib

    try:
        from antenv.axon_hooks import get_axon_ntff_profile_hook  # noqa: F401
        return
    except ImportError:
        pass

    import antenv

    mod = types.ModuleType("antenv.axon_hooks")
    mod._hook = None

    def set_axon_ntff_profile_hook(h):
        mod._hook = h

    def get_axon_ntff_profile_hook():
        return mod._hook

    mod.set_axon_ntff_profile_hook = set_axon_ntff_profile_hook
    mod.get_axon_ntff_profile_hook = get_axon_ntff_profile_hook
    sys.modules["antenv.axon_hooks"] = mod
    antenv.axon_hooks = mod

    so_path = "/opt/axon/libaxon_pjrt.so"
    try:
        lib = ctypes.CDLL(so_path)
    except OSError:
        return
    if not hasattr(lib, "axon_start_nrt_profile"):
        return
    lib.axon_start_nrt_profile.argtypes = [
        ctypes.POINTER(ctypes.c_int64),
        ctypes.c_size_t,
    ]
    lib.axon_start_nrt_profile.restype = ctypes.c_int64
    lib.axon_stop_nrt_profile.argtypes = [ctypes.c_char_p]
    lib.axon_stop_nrt_profile.restype = ctypes.c_int64

    @contextlib.contextmanager
    def _hook(output_dir, device_ids):
        import jax

        jax.devices()
        if device_ids:
            ids = (ctypes.c_int64 * len(device_ids))(*device_ids)
            rc = lib.axon_start_nrt_profile(ids, len(device_ids))
        else:
            rc = lib.axon_start_nrt_profile(None, 0)
        if rc != 0:
            raise RuntimeError(f"axon_start_nrt_profile rc={rc}")
        try:
            yield
        finally:
            n = lib.axon_stop_nrt_profile(str(output_dir).encode())
            if n < 0:
                raise RuntimeError(f"axon_stop_nrt_profile rc={n}")
            print(f"profile: {n} file(s) written to {output_dir}",
                  file=sys.stderr)

    set_axon_ntff_profile_hook(_hook)


def _build_nc():
    if "nc" in _nc_cache:
        return _nc_cache["nc"]
    import concourse.bass as bass
    import concourse.mybir as mybir

    f32 = mybir.dt.float32
    bf16 = mybir.dt.bfloat16
    nc = bass.Bass()
    pk_d = nc.declare_dram_parameter("pk", [48, PK_F], bf16, isOutput=False)
    out = nc.declare_dram_parameter("out", [OUT_ROWS, D], f32, isOutput=True)

    pk = nc.sbuf_tensor("pk_sb", [48, PK_F], bf16).__enter__()
    b_sb = nc.sbuf_tensor("b_sb", [128, CHUNK * D], f32).__enter__()
    warm = nc.sbuf_tensor("warm_sb", [1, 256], f32).__enter__()
    # PSUM: 4 rotating chunk slots of [128, 1024] f32 (2 banks each = all 8
    # banks).  mm1 writes cols 0:512 (bank A), mm2 cols 512:768 (bank B);
    # the copy engine (vector for even chunks, scalar for odd) reads both
    # bank-aligned halves.  A bank is never touched by two engines at once:
    # slot q%4 is written by PE only after the chunk q-4 copy signalled.
    accs = [nc.psum_tensor(f"acc{i}", [128, 1024], f32).__enter__()
            for i in range(4)]

    with (nc.Block() as block,
          nc.semaphore("dma_sem") as dma_sem,
          nc.semaphore("pe_sem") as pe_sem,
          nc.semaphore("v_sem") as v_sem,
          nc.semaphore("sc_sem") as sc_sem):

        @block.sync
        def _(sync):
            n = 0
            # two staged loads: tables + one-hot cols for chunks 0-1, then
            # the remaining one-hot cols.
            sync.dma_start(out=pk[:, 0:L1_COLS],
                           in_=pk_d[:, 0:L1_COLS]).then_inc(dma_sem, 16)
            sync.dma_start(out=pk[:, L1_COLS:],
                           in_=pk_d[:, L1_COLS:]).then_inc(dma_sem, 16)
            n += 32
            # replica 0: interleaved chunk-group writes (row j = 27*p + q).
            # Rows >= PERIOD carry wrap-correct content identical to what
            # replica 1 rewrites there, so no ordering between DMAs needed.
            for q0, g in GROUPS:
                qe = q0 + g
                sync.wait_ge(v_sem, (qe + 1) // 2)
                sync.wait_ge(sc_sem, qe // 2)
                dst = bass.AP(out, q0 * D, [[CHUNK * D, 128], [1, g * D]])
                sync.dma_start(out=dst,
                               in_=b_sb[:, q0 * D:qe * D]).then_inc(
                                   dma_sem, 16)
                n += 16
            # tail rows 16348..16383 = block rows 73..108 (wrap), on single
            # partitions; issued before the replicas so the few engines
            # involved absorb them while 40 MB of replica work still queues
            sync.dma_start(out=out[16348:16353, :],
                           in_=b_sb[2:3, 21 * D:26 * D]).then_inc(dma_sem, 16)
            sync.dma_start(out=out[16353:16379, :],
                           in_=b_sb[3:4, :]).then_inc(dma_sem, 16)
            sync.dma_start(out=out[16379:16384, :],
                           in_=b_sb[4:5, 0:5 * D]).then_inc(dma_sem, 16)
            n += 48
            # replicas 1..4, sequential DRAM regions on the one queue (a
            # second concurrent HWDGE queue halves HBM write throughput)
            for k in range(1, NREPS):
                base = k * PERIOD
                sync.dma_start(out=out[base:base + BROWS, :],
                               in_=b_sb[:]).then_inc(dma_sem, 16)
                n += 16
            sync.wait_ge(dma_sem, n)

        @block.tensor
        def _(tensor):
            # mm order within a chunk is level/time THEN const/exch/pair;
            # the copy engine reads in the same order, so each PSUM bank
            # gets >=0.5us of settle time after its matmul completion inc
            # (a copy released directly on a matmul's completion inc can
            # still read zeros from the not-yet-drained PSUM writes).
            tensor.wait_ge(dma_sem, 16)
            for q in range(CHUNK):
                if q == 2:
                    tensor.wait_ge(dma_sem, 32)
                if q >= 4:
                    prev = q - 4  # slot q%4 free once chunk q-4 was copied
                    if prev % 2 == 0:
                        tensor.wait_ge(v_sem, prev // 2 + 1)
                    else:
                        tensor.wait_ge(sc_sem, (prev + 1) // 2)
                acc = accs[q % 4]
                tensor.matmul(acc[:, 512:768],
                              pk[0:48, _ohb_off(q):_ohb_off(q) + 128],
                              pk[0:48, 512:768],
                              skip_group_check=True).then_inc(pe_sem)
                tensor.matmul(acc[:, 0:512],
                              pk[0:12, _oha_off(q):_oha_off(q) + 128],
                              pk[0:12, 0:512],
                              skip_group_check=True).then_inc(pe_sem)

        @block.vector
        def _(vector):
            for q in range(0, CHUNK, 2):
                vector.wait_ge(pe_sem, 2 * (q + 1))
                acc = accs[q % 4]
                vector.tensor_copy(b_sb[:, q * D + 512:(q + 1) * D],
                                   acc[:, 512:768])
                vector.tensor_copy(b_sb[:, q * D:q * D + 512],
                                   acc[:, 0:512]).then_inc(v_sem)

        @block.scalar
        def _(scalar):
            # dummy copy: triggers the one-time ACT_TABLE_LOAD (~1.3us) at
            # program start instead of on the first chunk's critical path
            scalar.copy(warm[:, 128:256], warm[:, 0:128])
            for q in range(1, CHUNK, 2):
                scalar.wait_ge(pe_sem, 2 * (q + 1))
                acc = accs[q % 4]
                scalar.copy(b_sb[:, q * D + 512:(q + 1) * D],
                            acc[:, 512:768])
                scalar.copy(b_sb[:, q * D:q * D + 512],
                            acc[:, 0:512]).then_inc(sc_sem)

    _nc_cache["nc"] = nc
    return nc


def _core_inputs(c, price_w, size_w, exchange_w, pair_w, level_w, time_w):
    import ml_dtypes

    phi = (c * RPC) % PERIOD
    pk = np.zeros((48, PK_F), np.float32)
    pk[0, 0:128] = price_w[0]
    pk[1, 128:256] = size_w[0]
    for j in range(3):
        pk[2 + j, 256:384] = exchange_w[j]
    for j in range(7):
        pk[5 + j, 384:512] = pair_w[j]
    for j in range(15):
        pk[j, 512:640] = level_w[j]
    for j in range(31):
        pk[15 + j, 640:768] = time_w[j]
    p = np.arange(128)
    for q in range(CHUNK):
        r = phi + CHUNK * p + q
        a, b = _oha_off(q), _ohb_off(q)
        pk[0, a + p] = 1.0
        pk[1, a + p] = 1.0
        pk[2 + r % 3, a + p] = 1.0
        pk[5 + r % 7, a + p] = 1.0
        pk[r % 15, b + p] = 1.0
        pk[15 + r % 31, b + p] = 1.0
    return {"pk": pk.astype(ml_dtypes.bfloat16)}


def kernel(price_w, size_w, exchange_w, pair_w, level_w, time_w,
           num_features=N):
    global LAST_EXEC_NS, LAST_RESULT
    assert int(num_features) == N

    from concourse.bass_utils import run_bass_kernel_spmd

    args = [np.asarray(x, np.float32) for x in
            (price_w, size_w, exchange_w, pair_w, level_w, time_w)]
    in_maps = [_core_inputs(c, *args) for c in range(NCORES)]

    if TRACE:
        _ensure_ntff_hook()
    nc = _build_nc()
    res = None
    for attempt in range(3):
        try:
            res = run_bass_kernel_spmd(nc, in_maps, list(range(NCORES)),
                                       trace=TRACE)
            break
        except Exception:
            if attempt == 2:
                raise
    LAST_EXEC_NS = res.exec_time_ns
    LAST_RESULT = res
    return np.concatenate([res.results[c]["out"][:RPC]
                           for c in range(NCORES)], axis=0)


# revision 19
# speedup vs baseline: 1.1702x; 1.1702x over previous
"""Trainium2 kernel for nn_H100SmartEmbedding (embedding_lookup).

Output [131072, 768] f32: cols 0:128 price_w[0] (const), 128:256 size_w[0]
(const), 256:384 exchange_w[i%3], 384:512 pair_w[i%7], 512:640 level_w[i%15],
640:768 time_w[i%31].  Rows repeat with period lcm(3,7,15,31)=3255.

Each of the 8 cores covers 16384 output rows.  The core builds one period
block (3328 rows = 128 partitions x 26 rows) in SBUF and replicates it to
DRAM with large contiguous writes; the ~48 MiB/core write is the memory
roofline (~25.5 GB/s x 16 SDMA engines after the 2-packets-per-79KB-line
turnaround tax).  Tables are bf16 (single component, rel err ~4e-3 vs the
2e-2 gate), so all six tables stack block-diagonally into K=12
(const/exch/pair) and K=48 (level/time) contraction rows: two matmuls +
two PSUM->SBUF copies per chunk, copies alternating vector/scalar per
chunk.  Hard-won constraints baked in here:
 - every big DMA covers all 128 partitions (a 121-partition DMA lands on
   just 11 SDMA engines and halves write throughput);
 - one HWDGE queue only (two concurrent queues collapsed throughput);
 - a PSUM region is copied only after one LATER matmul also completed --
   a copy released directly on its own matmul's completion semaphore
   races the PSUM write drain and intermittently reads zeros;
 - the 36 wrap tail rows go out as 3 tiny early-issued DMAs absorbed
   while the big replicas still queue.
"""

import sys

if "/opt/trn_rl_repo" not in sys.path:
    sys.path.insert(0, "/opt/trn_rl_repo")

import numpy as np

N = 131072
D = 768
PERIOD = 3255  # lcm(3, 7, 15, 31)
NCORES = 8
RPC = N // NCORES  # 16384 rows per core
CHUNK = 26  # rows per SBUF partition
NREPS = 5  # replica bases k*PERIOD, k=0..4
BROWS = CHUNK * 128  # 3328 staged rows (period + wrap-correct padding)
# Every big write uses all 128 partitions: a partial-partition DMA is
# spread over fewer SDMA engines (121 partitions -> 11 engines) and tanks
# HBM write throughput.  Replicas land at k*PERIOD writing the full block;
# rows 16348:16384 (block rows 73..108 by wrap) come from 3 tiny
# single-partition tail DMAs issued EARLY so their packets drain while the
# big replicas still queue behind them.
OUT_ROWS = RPC  # 16384, no padding
# Write groups of 2 chunks: production (~1.7us/pair) slightly outpaces the
# DMA service time per pair, so the write queue stays non-empty through the
# whole fill phase.
GROUPS = [(q0, 2) for q0 in range(0, CHUNK, 2)]

# packed input tensor pk [48, PK_F] bf16 free-dim layout:
#   [0:512]    tabsA rows 0:12 (price|size|exchange|pair block-diag)
#   [512:768]  tabsB rows 0:48 (level|time block-diag)
#   then one 512-col block per chunk pair j (chunks 2j, 2j+1):
#     [base + (q%2)*128]        ohA cols of chunk q  (rows 0:12)
#     [base + 256 + (q%2)*128]  ohB cols of chunk q  (rows 0:48)
# The first load slice [0:L1_COLS] unlocks chunks 0-1, the rest arrives in
# a second DMA that overlaps the first matmuls.
PK_F = 768 + 512 * (CHUNK // 2)  # 7424
L1_COLS = 1280


def _oha_off(q):
    return 768 + 512 * (q // 2) + (q % 2) * 128


def _ohb_off(q):
    return 768 + 512 * (q // 2) + 256 + (q % 2) * 128

TRACE = False
LAST_EXEC_NS = None
LAST_RESULT = None

_nc_cache = {}


def _ensure_ntff_hook():
    """The agent image's antenv package lacks axon_hooks, so the boot shim
    never registers the NTFF profile hook and trace=True crashes on import.
    Recreate the module + ctypes hook here (same recipe as trn_boot.py)."""
    import types
    import ctypes
    import contextlib

    try:
        from antenv.axon_hooks import get_axon_ntff_profile_hook  # noqa: F401
        return
    except ImportError:
        pass

    import antenv

    mod = types.ModuleType("antenv.axon_hooks")
    mod._hook = None

    def set_axon_ntff_profile_hook(h):
        mod._hook = h

    def get_axon_ntff_profile_hook():
        return mod._hook

    mod.set_axon_ntff_profile_hook = set_axon_ntff_profile_hook
    mod.get_axon_ntff_profile_hook = get_axon_ntff_profile_hook
    sys.modules["antenv.axon_hooks"] = mod
    antenv.axon_hooks = mod

    so_path = "/opt/axon/libaxon_pjrt.so"
    try:
        lib = ctypes.CDLL(so_path)
    except OSError:
        return
    if not hasattr(lib, "axon_start_nrt_profile"):
        return
    lib.axon_start_nrt_profile.argtypes = [
        ctypes.POINTER(ctypes.c_int64),
        ctypes.c_size_t,
    ]
    lib.axon_start_nrt_profile.restype = ctypes.c_int64
    lib.axon_stop_nrt_profile.argtypes = [ctypes.c_char_p]
    lib.axon_stop_nrt_profile.restype = ctypes.c_int64

    @contextlib.contextmanager
    def _hook(output_dir, device_ids):
        import jax

        jax.devices()
        if device_ids:
            ids = (ctypes.c_int64 * len(device_ids))(*device_ids)
            rc = lib.axon_start_nrt_profile(ids, len(device_ids))
        else:
            rc = lib.axon_start_nrt_profile(None, 0)
        if rc != 0:
            raise RuntimeError(f"axon_start_nrt_profile rc={rc}")
        try:
            yield
        finally:
            n = lib.axon_stop_nrt_profile(str(output_dir).encode())
            if n < 0:
                raise RuntimeError(f"axon_stop_nrt_profile rc={n}")
            print(f"profile: {n} file(s) written to {output_dir}",
                  file=sys.stderr)

    set_axon_ntff_profile_hook(_hook)


def _build_nc():
    if "nc" in _nc_cache:
        return _nc_cache["nc"]
    import concourse.bass as bass
    import concourse.mybir as mybir

    f32 = mybir.dt.float32
    bf16 = mybir.dt.bfloat16
    nc = bass.Bass()
    pk_d = nc.declare_dram_parameter("pk", [48, PK_F], bf16, isOutput=False)
    out = nc.declare_dram_parameter("out", [OUT_ROWS, D], f32, isOutput=True)

    pk = nc.sbuf_tensor("pk_sb", [48, PK_F], bf16).__enter__()
    b_sb = nc.sbuf_tensor("b_sb", [128, CHUNK * D], f32).__enter__()
    warm = nc.sbuf_tensor("warm_sb", [1, 256], f32).__enter__()
    # PSUM: 4 rotating chunk slots of [128, 1024] f32 (2 banks each = all 8
    # banks).  mm1 writes cols 0:512 (bank A), mm2 cols 512:768 (bank B);
    # the copy engine (vector for even chunks, scalar for odd) reads both
    # bank-aligned halves.  A bank is never touched by two engines at once:
    # slot q%4 is written by PE only after the chunk q-4 copy signalled.
    accs = [nc.psum_tensor(f"acc{i}", [128, 1024], f32).__enter__()
            for i in range(4)]

    with (nc.Block() as block,
          nc.semaphore("dma_sem") as dma_sem,
          nc.semaphore("pe_sem") as pe_sem,
          nc.semaphore("v_sem") as v_sem,
          nc.semaphore("sc_sem") as sc_sem):

        @block.sync
        def _(sync):
            n = 0
            # two staged loads: tables + one-hot cols for chunks 0-1, then
            # the remaining one-hot cols.
            sync.dma_start(out=pk[:, 0:L1_COLS],
                           in_=pk_d[:, 0:L1_COLS]).then_inc(dma_sem, 16)
            sync.dma_start(out=pk[:, L1_COLS:],
                           in_=pk_d[:, L1_COLS:]).then_inc(dma_sem, 16)
            n += 32
            # replica 0: interleaved chunk-group writes (row j = 27*p + q).
            # Rows >= PERIOD carry wrap-correct content identical to what
            # replica 1 rewrites there, so no ordering between DMAs needed.
            for q0, g in GROUPS:
                qe = q0 + g
                sync.wait_ge(v_sem, (qe + 1) // 2)
                sync.wait_ge(sc_sem, qe // 2)
                dst = bass.AP(out, q0 * D, [[CHUNK * D, 128], [1, g * D]])
                sync.dma_start(out=dst,
                               in_=b_sb[:, q0 * D:qe * D]).then_inc(
                                   dma_sem, 16)
                n += 16
            # tail rows 16348..16383 = block rows 73..108 (wrap), on single
            # partitions; issued before the replicas so the few engines
            # involved absorb them while 40 MB of replica work still queues
            sync.dma_start(out=out[16348:16353, :],
                           in_=b_sb[2:3, 21 * D:26 * D]).then_inc(dma_sem, 16)
            sync.dma_start(out=out[16353:16379, :],
                           in_=b_sb[3:4, :]).then_inc(dma_sem, 16)
            sync.dma_start(out=out[16379:16384, :],
                           in_=b_sb[4:5, 0:5 * D]).then_inc(dma_sem, 16)
            n += 48
            # replicas 1..4, sequential DRAM regions on the one queue (a
            # second concurrent HWDGE queue halves HBM write throughput)
            for k in range(1, NREPS):
                base = k * PERIOD
                sync.dma_start(out=out[base:base + BROWS, :],
                               in_=b_sb[:]).then_inc(dma_sem, 16)
                n += 16
            sync.wait_ge(dma_sem, n)

        @block.tensor
        def _(tensor):
            # mm order within a chunk is level/time THEN const/exch/pair;
            # the copy engine reads in the same order, so each PSUM bank
            # gets >=0.5us of settle time after its matmul completion inc
            # (a copy released directly on a matmul's completion inc can
            # still read zeros from the not-yet-drained PSUM writes).
            tensor.wait_ge(dma_sem, 16)
            for q in range(CHUNK):
                if q == 2:
                    tensor.wait_ge(dma_sem, 32)
                if q >= 4:
                    prev = q - 4  # slot q%4 free once chunk q-4 was copied
                    if prev % 2 == 0:
                        tensor.wait_ge(v_sem, prev // 2 + 1)
                    else:
                        tensor.wait_ge(sc_sem, (prev + 1) // 2)
                acc = accs[q % 4]
                tensor.matmul(acc[:, 512:768],
                              pk[0:48, _ohb_off(q):_ohb_off(q) + 128],
                              pk[0:48, 512:768],
                              skip_group_check=True).then_inc(pe_sem)
                tensor.matmul(acc[:, 0:512],
                              pk[0:12, _oha_off(q):_oha_off(q) + 128],
                              pk[0:12, 0:512],
                              skip_group_check=True).then_inc(pe_sem)

        @block.vector
        def _(vector):
            for q in range(0, CHUNK, 2):
                vector.wait_ge(pe_sem, 2 * (q + 1))
                acc = accs[q % 4]
                vector.tensor_copy(b_sb[:, q * D + 512:(q + 1) * D],
                                   acc[:, 512:768])
                vector.tensor_copy(b_sb[:, q * D:q * D + 512],
                                   acc[:, 0:512]).then_inc(v_sem)

        @block.scalar
        def _(scalar):
            # dummy copy: triggers the one-time ACT_TABLE_LOAD (~1.3us) at
            # program start instead of on the first chunk's critical path
            scalar.copy(warm[:, 128:256], warm[:, 0:128])
            for q in range(1, CHUNK, 2):
                scalar.wait_ge(pe_sem, 2 * (q + 1))
                acc = accs[q % 4]
                scalar.copy(b_sb[:, q * D + 512:(q + 1) * D],
                            acc[:, 512:768])
                scalar.copy(b_sb[:, q * D:q * D + 512],
                            acc[:, 0:512]).then_inc(sc_sem)

    _nc_cache["nc"] = nc
    return nc


def _core_inputs(c, price_w, size_w, exchange_w, pair_w, level_w, time_w):
    import ml_dtypes

    phi = (c * RPC) % PERIOD
    pk = np.zeros((48, PK_F), np.float32)
    pk[0, 0:128] = price_w[0]
    pk[1, 128:256] = size_w[0]
    for j in range(3):
        pk[2 + j, 256:384] = exchange_w[j]
    for j in range(7):
        pk[5 + j, 384:512] = pair_w[j]
    for j in range(15):
        pk[j, 512:640] = level_w[j]
    for j in range(31):
        pk[15 + j, 640:768] = time_w[j]
    p = np.arange(128)
    for q in range(CHUNK):
        r = phi + CHUNK * p + q
        a, b = _oha_off(q), _ohb_off(q)
        pk[0, a + p] = 1.0
        pk[1, a + p] = 1.0
        pk[2 + r % 3, a + p] = 1.0
        pk[5 + r % 7, a + p] = 1.0
        pk[r % 15, b + p] = 1.0
        pk[15 + r % 31, b + p] = 1.0
    return {"pk": pk.astype(ml_dtypes.bfloat16)}


def kernel(price_w, size_w, exchange_w, pair_w, level_w, time_w,
           num_features=N):
    global LAST_EXEC_NS, LAST_RESULT
    assert int(num_features) == N

    from concourse.bass_utils import run_bass_kernel_spmd

    args = [np.asarray(x, np.float32) for x in
            (price_w, size_w, exchange_w, pair_w, level_w, time_w)]
    in_maps = [_core_inputs(c, *args) for c in range(NCORES)]

    if TRACE:
        _ensure_ntff_hook()
    nc = _build_nc()
    res = None
    for attempt in range(3):
        try:
            res = run_bass_kernel_spmd(nc, in_maps, list(range(NCORES)),
                                       trace=TRACE)
            break
        except Exception:
            if attempt == 2:
                raise
    LAST_EXEC_NS = res.exec_time_ns
    LAST_RESULT = res
    return np.concatenate([res.results[c]["out"][:RPC]
                           for c in range(NCORES)], axis=0)
